# revision 14
# baseline (speedup 1.0000x reference)
"""Two-layer GCN (PyG GCNConv x2 + ReLU) on 8 Trainium2 NeuronCores.

Strategy (graph/data parallel, destination-partitioned edges):
  - Nodes row-sharded across 8 cores (6250 real + pad -> 6272 per core).
  - Layer 1: input staged host-side in EDGE-SLOT order, each slot =
    x[src]*dinv[src]*dinv[dst] (dst scale folded host-side - every slot
    has exactly one dst), incl. self-loop slots.  Per dst tile: PE
    scatter-matmuls feature slots against an on-chip one-hot S into
    PSUM, then a W1 GEMM + ACT Relu.  S built with broadcast IS_EQ
    groups split between Vector and GpSimd (GpSimd is idle in layer 1).
  - Node rows are staged to DRAM per tile and AllGather'ed in FOUR
    node-pieces so collectives and layer-2 gathers pipeline with the
    tail of layer 1.
  - Layer 2: per-edge source rows fetched with SWDGE dma_gather
    (gen_mode=0) round-robin on 4 queues - transfers overlap across
    queues; the engine slice is residual wait.  prepare_only mode is a
    trap: its desc-gen alone runs at ~8ns/desc serial on Q7.  Four
    passes (one per source piece) accumulate via PSUM + an f16 acc
    buffer (identity-matmul re-inject); the self-loop term is one extra
    matmul (W2^T (h1*dinv)) in pass 0; the final dinv[dst] scale is a
    per-tile DVE multiply before the ACT Relu.
  - fp16 operands with fp32 PSUM accumulation.
"""

import os
import sys

import numpy as np

for _p in ("/opt/trn_rl_repo", "/root/.axon_site/_ro/trn_rl_repo"):
    if os.path.isdir(_p) and _p not in sys.path:
        sys.path.append(_p)

import concourse.bacc as bacc
import concourse.bass as bass
import concourse.mybir as mybir
import concourse.tile as tile
from concourse.bass_utils import run_bass_kernel_spmd

# Problem constants (hardcoded per harness contract).
N, E, IN, HID, OUT = 50000, 800000, 128, 128, 64
NCORES = 8
NPC_REAL = N // NCORES          # 6250
TILES = 49
NPC = TILES * 128               # 6272 padded nodes per core
PIECE_T = [17, 16, 16]          # tiles per AllGather piece
NPIECES = len(PIECE_T)
PIECE_T0 = [0, 17, 33]          # first tile of each piece
PIECE_H = [t * 128 for t in PIECE_T]           # rows per core per piece
PIECE_OFF = [t0 * 128 for t0 in PIECE_T0]      # row offset within core
WV = 16                         # gather wave size in chunks (128 slots each)
WVS = 16                        # S-build group size in chunks
NQ = 4


def default_cfg():
    return dict(N=N, E=E, IN=IN, HID=HID, OUT=OUT, NCORES=NCORES,
                NPC_REAL=NPC_REAL, TILES=TILES, NPC=NPC, WV=WV)

F16 = mybir.dt.float16
F32 = mybir.dt.float32
NPF16 = np.float16

_ts = bass.ts


def _schedule(owner, tile_id, key, nregions, TILES, NCORES, tiebreak=None):
    """Chunk schedule for edges grouped by (owner, tile, region)."""
    cnt = np.zeros((NCORES, TILES, nregions), np.int64)
    np.add.at(cnt, (owner, tile_id, key), 1)
    K = np.ceil(cnt.max(axis=0) / 128).astype(np.int64)
    Kr = [K[:, r].copy() for r in range(nregions)]
    Cr = [int(k.sum()) for k in Kr]
    bases = []
    off = 0
    for r in range(nregions):
        b = off + np.concatenate([[0], np.cumsum(Kr[r])[:-1]]).astype(np.int64)
        bases.append(b)
        off += Cr[r]
    C = off

    gid = (owner * TILES + tile_id) * nregions + key
    if tiebreak is None:
        tiebreak = np.arange(len(gid))
    order = np.lexsort((tiebreak, gid))
    gs = gid[order]
    starts = np.concatenate([[0], np.flatnonzero(np.diff(gs)) + 1])
    group_of = np.searchsorted(starts, np.arange(len(gs)), side="right") - 1
    pos = np.arange(len(gs)) - starts[group_of]

    base_chunk = np.empty(len(gid), np.int64)
    for r in range(nregions):
        m = key == r
        base_chunk[m] = bases[r][tile_id[m]]
    slot = np.empty(len(gs), np.int64)
    slot[order] = base_chunk[order] * 128 + pos
    return dict(K=Kr, C=Cr, bases=bases, Ctot=C, slot=slot)


def _preprocess(edge_index: np.ndarray, cfg=None):
    g = cfg or default_cfg()
    N, NCORES, NPC_REAL, TILES = g["N"], g["NCORES"], g["NPC_REAL"], g["TILES"]
    src = np.asarray(edge_index[0], np.int64)
    dst = np.asarray(edge_index[1], np.int64)
    deg = np.bincount(dst, minlength=N).astype(np.float64) + 1.0
    dinv_n = (1.0 / np.sqrt(deg)).astype(np.float32)

    # ---- layer 1: edges + self-loops, single region, slots carry x[src] ----
    selfn = np.arange(N, dtype=np.int64)
    src1 = np.concatenate([src, selfn])
    dst1 = np.concatenate([dst, selfn])
    own1 = dst1 // NPC_REAL
    dl1 = dst1 % NPC_REAL
    t1 = dl1 // 128
    it1 = dl1 % 128
    s1 = _schedule(own1, t1, np.zeros(len(src1), np.int64), 1, TILES,
                   NCORES, tiebreak=src1)
    C1 = s1["Ctot"]
    dstid1 = np.full((NCORES, C1 * 128), -1.0, np.float32)
    dstid1[own1, s1["slot"]] = it1
    dst1_t = np.ascontiguousarray(
        dstid1.reshape(NCORES, C1, 128).transpose(0, 2, 1)).astype(NPF16)
    srcof1 = np.full((NCORES, C1 * 128), -1, np.int64)
    srcof1[own1, s1["slot"]] = src1
    # dinv[dst] per layer-1 slot, folded host-side into the xgs slot values
    dinvdst1 = np.zeros((NCORES, C1 * 128), np.float32)
    dinvdst1[own1, s1["slot"]] = dinv_n[dst1]

    # ---- layer 2: edges only, regions = source node piece ----
    own2 = dst // NPC_REAL
    dl2 = dst % NPC_REAL
    t2 = dl2 // 128
    it2 = dl2 % 128
    srem = src % NPC_REAL
    offs = np.asarray(PIECE_OFF + [NPC], np.int64)
    reg2 = (np.searchsorted(offs, srem, side="right") - 1).astype(np.int64)
    hp = np.asarray(PIECE_H, np.int64)
    rowab = (src // NPC_REAL) * hp[reg2] + (srem - offs[reg2])
    s2 = _schedule(own2, t2, reg2, NPIECES, TILES, NCORES, tiebreak=rowab)
    C2 = s2["Ctot"]
    idx16 = np.zeros((NCORES, C2 * 128), np.int16)
    dstid2 = np.full((NCORES, C2 * 128), -1.0, np.float32)
    idx16[own2, s2["slot"]] = rowab.astype(np.int16)
    dstid2[own2, s2["slot"]] = it2
    idx_t = idx16.reshape(NCORES, C2 * 8, 16).transpose(0, 2, 1)
    idx_t = np.tile(idx_t, (1, 8, 1)).copy()                    # [8,128,C2*8]
    dst2_t = np.ascontiguousarray(
        dstid2.reshape(NCORES, C2, 128).transpose(0, 2, 1)).astype(NPF16)

    return dict(deg=deg, dinv_n=dinv_n, C1=C1, K1=s1["K"][0], B1=s1["bases"][0],
                dst1_t=dst1_t, srcof1=srcof1, dinvdst1=dinvdst1,
                C2=C2, K2=s2["K"], C2r=s2["C"], B2=s2["bases"],
                idx_t=idx_t, dst2_t=dst2_t)


def _waves(n_chunks: int, chunk0: int, wv: int):
    out, c = [], 0
    while c < n_chunks:
        n = min(wv, n_chunks - c)
        out.append((chunk0 + c, n))
        c += n
    return out


def _build_program(meta, cfg=None):
    g = cfg or default_cfg()
    IN, HID, OUT = g["IN"], g["HID"], g["OUT"]
    NCORES, TILES, NPC = g["NCORES"], g["TILES"], g["NPC"]
    C1, K1, B1 = meta["C1"], meta["K1"], meta["B1"]
    C2, K2, C2r, B2 = meta["C2"], meta["K2"], meta["C2r"], meta["B2"]

    nc = bacc.Bacc("TRN2", target_bir_lowering=False, debug=False,
                   num_devices=NCORES, num_swdge_queues=NQ)

    # ---- I/O ----
    xgs_d = nc.dram_tensor("xgs", [128, C1 * 128], F16, kind="ExternalInput")
    w1_d = nc.dram_tensor("W1", [IN, HID], F16, kind="ExternalInput")
    w2_d = nc.dram_tensor("W2", [HID, OUT], F16, kind="ExternalInput")
    b1_d = nc.dram_tensor("b1c", [HID, 1], F32, kind="ExternalInput")
    b2_d = nc.dram_tensor("b2c", [OUT, 1], F32, kind="ExternalInput")
    dinvrep_d = nc.dram_tensor("dinvrep", [128, NPC], F16,
                               kind="ExternalInput")
    ident_d = nc.dram_tensor("ident", [128, 128], F16, kind="ExternalInput")
    idx_d = nc.dram_tensor("idxt", [128, C2 * 8], mybir.dt.int16,
                           kind="ExternalInput")
    dst1_d = nc.dram_tensor("dstt1", [128, C1], F16, kind="ExternalInput")
    dst2_d = nc.dram_tensor("dstt2", [128, C2], F16, kind="ExternalInput")
    out_d = nc.dram_tensor("outT", [OUT, NPC], F32, kind="ExternalOutput")

    gdram = [nc.dram_tensor(f"gdram{p}", [PIECE_H[p], 128], F16)
             for p in range(NPIECES)]
    tables = [nc.dram_tensor(f"table{p}", [NCORES * PIECE_H[p], 128], F16,
                             addr_space="Shared") for p in range(NPIECES)]
    rg = [list(range(NCORES))]

    with tile.TileContext(nc) as tc:
        with (
            tc.tile_pool(name="const", bufs=1) as constp,
            tc.tile_pool(name="rdp", bufs=1) as rdp,
            tc.tile_pool(name="accp", bufs=1) as accp,
            tc.tile_pool(name="outp", bufs=3) as outp,
            tc.tile_pool(name="xw", bufs=3) as xwp,
            tc.tile_pool(name="gt", bufs=16) as gtp,
            tc.tile_pool(name="s1", bufs=3) as s1p,
            tc.tile_pool(name="s2", bufs=3) as s2p,
            tc.tile_pool(name="sx", bufs=3) as sxp,
            tc.tile_pool(name="rt", bufs=3) as rtp,
            tc.tile_pool(name="stg", bufs=3) as stgp,
            tc.tile_pool(name="psx", bufs=2, space="PSUM") as psxp,
            tc.tile_pool(name="pgem", bufs=2, space="PSUM") as pgemp,
            tc.tile_pool(name="pg2", bufs=1, space="PSUM") as pg2p,
            tc.tile_pool(name="psc", bufs=3, space="PSUM") as pscp,
        ):
            # ---- constants ----
            w1 = constp.tile([IN, HID], F16, tag="w1")
            nc.sync.dma_start(w1[:], w1_d[:, :])
            w2 = constp.tile([HID, OUT], F16, tag="w2")
            nc.sync.dma_start(w2[:], w2_d[:, :])
            b1 = constp.tile([HID, 1], F32, tag="b1")
            nc.sync.dma_start(b1[:], b1_d[:, :])
            b2 = constp.tile([OUT, 1], F32, tag="b2")
            nc.sync.dma_start(b2[:], b2_d[:, :])
            ident = constp.tile([128, 128], F16, tag="ident")
            nc.sync.dma_start(ident[:], ident_d[:, :])
            idxt = constp.tile([128, C2 * 8], mybir.dt.int16, tag="idxt")
            nc.sync.dma_start(idxt[:], idx_d[:, :])
            dstt1 = constp.tile([128, C1], F16, tag="dstt1")
            nc.sync.dma_start(dstt1[:], dst1_d[:, :])
            dstt2 = constp.tile([128, C2], F16, tag="dstt2")
            nc.sync.dma_start(dstt2[:], dst2_d[:, :])
            dinvrep = constp.tile([128, NPC], F16, tag="dinvrep")
            nc.sync.dma_start(dinvrep[:], dinvrep_d[:, :])

            iotat = constp.tile([128, WVS * 128], F16, tag="iotat")
            nc.gpsimd.iota(iotat[:].rearrange("p (k j) -> p k j", j=128),
                           [[0, WVS], [1, 128]], channel_multiplier=0,
                           allow_small_or_imprecise_dtypes=True)

            def build_s(eng, st, dstt, c0, n):
                eng.tensor_tensor(
                    st[:, :n * 128].rearrange("p (k j) -> p k j", j=128),
                    iotat[:, :n * 128].rearrange("p (k j) -> p k j", j=128),
                    dstt[:, c0:c0 + n].rearrange("p (k o) -> p k o", o=1)
                        .to_broadcast([128, n, 128]),
                    mybir.AluOpType.is_equal)

            # =================== LAYER 1 + piece staging ====================
            xgs_view = xgs_d.ap().rearrange("p (c f) -> p c f", f=128)
            l1_waves = _waves(C1, 0, WV)
            l1_sgrps = _waves(C1, 0, WVS)
            wave1, s1_tiles = {}, {}

            def ensure_wave1(wi):
                if wi in wave1:
                    return wave1[wi]
                c0, n = l1_waves[wi]
                t = xwp.tile([128, WV, 128], F16, tag="xw")
                nc.sync.dma_start(t[:, :n, :], xgs_view[:, c0:c0 + n, :])
                wave1[wi] = t
                return t

            def ensure_s1(wi):
                if wi in s1_tiles:
                    return s1_tiles[wi]
                c0, n = l1_sgrps[wi]
                st = s1p.tile([128, WVS * 128], F16, tag="s1")
                build_s(nc.vector, st, dstt1, c0, n)
                s1_tiles[wi] = st
                return st

            rd = rdp.tile([128, NPC], F16, tag="rd")       # h1 * dinv

            gviews = [gdram[p].ap().rearrange("(t p) f -> p t f", p=128)
                      for p in range(NPIECES)]  # f = OUT (64)
            piece_of = []
            for p in range(NPIECES):
                piece_of += [p] * PIECE_T[p]

            for t in range(TILES):
                nch = int(K1[t])
                sl = _ts(t, 128)
                psx = psxp.tile([IN, 128], F32, tag="psx")
                for k in range(nch):
                    ch = int(B1[t]) + k
                    xg = ensure_wave1(ch // WV)
                    sw = ensure_s1(ch // WVS)
                    pos, spos = ch % WV, ch % WVS
                    nc.tensor.matmul(
                        psx[:IN, :], xg[:, pos, :IN],
                        sw[:, spos * 128:(spos + 1) * 128],
                        start=(k == 0), stop=(k == nch - 1))
                sx = sxp.tile([IN, 128], F16, tag="sx")
                nc.scalar.copy(sx[:, :], psx[:IN, :])
                pz = pgemp.tile([128, 128], F32, tag="pgem")
                nc.tensor.matmul(pz[:HID, :], w1[:, :HID], sx[:, :],
                                 start=True, stop=True)
                # epilogue: rt = Relu(pz + b1) = h1 ; rd = h1*dinv
                rt = rtp.tile([128, 128], F16, tag="rt")
                nc.scalar.activation(
                    rt[:HID, :], pz[:HID, :],
                    mybir.ActivationFunctionType.Relu,
                    bias=b1[:HID, :], scale=1.0)
                nc.vector.tensor_tensor(
                    rd[:HID, sl], rt[:HID, :], dinvrep[:HID, sl],
                    mybir.AluOpType.mult)
                # stage this tile's node rows: (h1*dinv) @ W2, node-major
                ps2 = pg2p.tile([128, OUT], F32, tag="pg2")
                nc.tensor.matmul(ps2[:, :OUT], rd[:HID, sl], w2[:, :OUT],
                                 start=True, stop=True)
                stg = stgp.tile([128, OUT], F16, tag="stg")
                nc.scalar.copy(stg[:, :], ps2[:, :OUT])
                p = piece_of[t]
                nc.sync.dma_start(gviews[p][:, t - PIECE_T0[p], :OUT],
                                  stg[:, :])
                if t == PIECE_T0[p] + PIECE_T[p] - 1:
                    with tc.high_priority():
                        nc.gpsimd.collective_compute(
                            "AllGather", mybir.AluOpType.bypass,
                            replica_groups=rg,
                            ins=[gdram[p].ap()], outs=[tables[p].ap()])

            # =================== LAYER 2 gathers (prepare+trigger) ==========
            r_waves = [_waves(C2r[r], sum(C2r[:r]), WV) for r in range(NPIECES)]
            r_sgrps = [_waves(C2r[r], sum(C2r[:r]), WVS) for r in range(NPIECES)]
            wave2, s2_tiles = {}, {}
            qrr = [0]
            for r in range(NPIECES):
                with tc.tile_wait_until(0.02 + 0.01 * r):
                    for wi, (c0, n) in enumerate(r_waves[r]):
                        gt = gtp.tile([128, WV, 128], F16, tag="gt")
                        qn = qrr[0]
                        qrr[0] = (qn + 1) % NQ
                        nc.gpsimd.dma_gather(
                            gt[:, :n, :], tables[r][:, :],
                            idxt[:, c0 * 8:(c0 + n) * 8],
                            n * 128, n * 128, 128, single_packet=False,
                            queue_num=qn)
                        wave2[(r, wi)] = gt

            def ensure_s2(r, wi):
                key = (r, wi)
                if key in s2_tiles:
                    return s2_tiles[key]
                c0, n = r_sgrps[r][wi]
                st = s2p.tile([128, WVS * 128], F16, tag="s2")
                build_s(nc.vector, st, dstt2, c0, n)
                s2_tiles[key] = st
                return st

            acc = accp.tile([OUT, NPC], F16, tag="acc")

            # one pass per source piece; self term folded into pass 0
            for r in range(NPIECES):
                base0 = sum(C2r[:r])
                for t in range(TILES):
                    nch = int(K2[r][t])
                    sl = _ts(t, 128)
                    pscat = pscp.tile([OUT, 128], F32, tag="psc")
                    if r == 0:
                        nc.tensor.matmul(pscat[:OUT, :], w2[:, :OUT],
                                         rd[:HID, sl],
                                         start=True, stop=(nch == 0))
                    else:
                        nc.tensor.matmul(pscat[:OUT, :], ident[:OUT, :OUT],
                                         acc[:, sl],
                                         start=True, stop=(nch == 0))
                    for k in range(nch):
                        ch = int(B2[r][t]) + k
                        rel = ch - base0
                        gt = wave2[(r, rel // WV)]
                        sw = ensure_s2(r, rel // WVS)
                        pos, spos = rel % WV, rel % WVS
                        nc.tensor.matmul(
                            pscat[:OUT, :], gt[:, pos, :OUT],
                            sw[:, spos * 128:(spos + 1) * 128],
                            start=False, stop=(k == nch - 1))
                    if r < NPIECES - 1:
                        nc.scalar.copy(acc[:, sl], pscat[:OUT, :])
                    else:
                        tmp = rtp.tile([OUT, 128], F32, tag="ltmp")
                        nc.vector.tensor_tensor(
                            tmp[:, :], pscat[:OUT, :], dinvrep[:OUT, sl],
                            mybir.AluOpType.mult)
                        ot = outp.tile([OUT, 128], F32, tag="out")
                        nc.scalar.activation(
                            ot[:], tmp[:, :],
                            mybir.ActivationFunctionType.Relu,
                            bias=b2[:OUT, :], scale=1.0)
                        nc.sync.dma_start(out_d[:, sl], ot[:])

    nc.compile()
    return nc


def _host_inputs(inputs, meta, cfg=None):
    g = cfg or default_cfg()
    N, IN, HID, OUT = g["N"], g["IN"], g["HID"], g["OUT"]
    NCORES, NPC_REAL, NPC = g["NCORES"], g["NPC_REAL"], g["NPC"]
    x = np.asarray(inputs["x"], np.float32)
    W1 = np.asarray(inputs["W1"], np.float32)
    b1 = np.asarray(inputs["b1"], np.float32)
    W2 = np.asarray(inputs["W2"], np.float32)
    b2 = np.asarray(inputs["b2"], np.float32)
    C1 = meta["C1"]
    dinv_n = meta["dinv_n"]                                 # [N]
    xg = x * dinv_n[:, None]                                # [N, IN] f32

    ident = np.eye(128, dtype=NPF16)
    w1c = W1.astype(NPF16)
    w2c = W2.astype(NPF16)
    b1c = b1.reshape(HID, 1).astype(np.float32)
    b2c = b2.reshape(OUT, 1).astype(np.float32)

    in_maps = []
    for c in range(NCORES):
        srcof = meta["srcof1"][c]                           # [C1*128]
        xslots = np.zeros((C1 * 128, IN), NPF16)
        m = srcof >= 0
        xslots[m] = (xg[srcof[m]]
                     * meta["dinvdst1"][c][m, None]).astype(NPF16)
        xgs = np.ascontiguousarray(
            xslots.reshape(C1, 128, IN).transpose(1, 0, 2)
        ).reshape(128, C1 * IN)

        dl = np.ones(NPC, np.float32)
        dl[:NPC_REAL] = dinv_n[c * NPC_REAL:(c + 1) * NPC_REAL]
        dinvrep = np.tile(dl[None, :], (128, 1)).astype(NPF16)

        in_maps.append({
            "xgs": xgs, "W1": w1c, "W2": w2c, "b1c": b1c, "b2c": b2c,
            "dinvrep": dinvrep, "ident": ident,
            "idxt": meta["idx_t"][c],
            "dstt1": meta["dst1_t"][c], "dstt2": meta["dst2_t"][c],
        })
    return in_maps


def kernel(**inputs) -> np.ndarray:
    meta = _preprocess(np.asarray(inputs["edge_index"]))
    nc = _build_program(meta)
    in_maps = _host_inputs(inputs, meta)
    res = run_bass_kernel_spmd(nc, in_maps, list(range(NCORES)))
    out = np.empty((N, OUT), np.float32)
    for c in range(NCORES):
        out[c * NPC_REAL:(c + 1) * NPC_REAL] = \
            res.results[c]["outT"][:, :NPC_REAL].T
    return out


# revision 16
# speedup vs baseline: 1.0637x; 1.0637x over previous
"""Two-layer GCN (PyG GCNConv x2 + ReLU) on 8 Trainium2 NeuronCores.

Strategy (graph/data parallel, destination-partitioned edges):
  - Nodes row-sharded across 8 cores (6250 real + pad -> 6272 per core).
  - Layer 1: input staged host-side in EDGE-SLOT order, each slot =
    x[src]*dinv[src]*dinv[dst] (dst scale folded host-side - every slot
    has exactly one dst), incl. self-loop slots.  Per dst tile: PE
    scatter-matmuls feature slots against an on-chip one-hot S into
    PSUM, then a W1 GEMM + ACT Relu.  S built with broadcast IS_EQ
    groups split between Vector and GpSimd (GpSimd is idle in layer 1).
  - Node rows are staged to DRAM per tile and AllGather'ed in FOUR
    node-pieces so collectives and layer-2 gathers pipeline with the
    tail of layer 1.
  - Layer 2: per-edge source rows fetched with SWDGE dma_gather
    (gen_mode=0) round-robin on 4 queues - transfers overlap across
    queues; the engine slice is residual wait.  prepare_only mode is a
    trap: its desc-gen alone runs at ~8ns/desc serial on Q7.  Four
    passes (one per source piece) accumulate via PSUM + an f16 acc
    buffer (identity-matmul re-inject); the self-loop term is one extra
    matmul (W2^T (h1*dinv)) in pass 0; the final dinv[dst] scale is a
    per-tile DVE multiply before the ACT Relu.
  - fp16 operands with fp32 PSUM accumulation.
"""

import os
import sys

import numpy as np

for _p in ("/opt/trn_rl_repo", "/root/.axon_site/_ro/trn_rl_repo"):
    if os.path.isdir(_p) and _p not in sys.path:
        sys.path.append(_p)

import concourse.bacc as bacc
import concourse.bass as bass
import concourse.mybir as mybir
import concourse.tile as tile
from concourse.bass_utils import run_bass_kernel_spmd

# Problem constants (hardcoded per harness contract).
N, E, IN, HID, OUT = 50000, 800000, 128, 128, 64
NCORES = 8
NPC_REAL = N // NCORES          # 6250
TILES = 49
NPC = TILES * 128               # 6272 padded nodes per core
PIECE_T = [17, 16, 16]          # tiles per AllGather piece
NPIECES = len(PIECE_T)
PIECE_T0 = [0, 17, 33]          # first tile of each piece
PIECE_H = [t * 128 for t in PIECE_T]           # rows per core per piece
PIECE_OFF = [t0 * 128 for t0 in PIECE_T0]      # row offset within core
WV = 32                         # gather wave size in chunks (128 slots each)
WVS = 16                        # S-build group size in chunks
NQ = 4


def default_cfg():
    return dict(N=N, E=E, IN=IN, HID=HID, OUT=OUT, NCORES=NCORES,
                NPC_REAL=NPC_REAL, TILES=TILES, NPC=NPC, WV=WV)

F16 = mybir.dt.float16
F32 = mybir.dt.float32
NPF16 = np.float16

_ts = bass.ts


def _schedule(owner, tile_id, key, nregions, TILES, NCORES, tiebreak=None):
    """Chunk schedule for edges grouped by (owner, tile, region)."""
    cnt = np.zeros((NCORES, TILES, nregions), np.int64)
    np.add.at(cnt, (owner, tile_id, key), 1)
    K = np.ceil(cnt.max(axis=0) / 128).astype(np.int64)
    Kr = [K[:, r].copy() for r in range(nregions)]
    Cr = [int(k.sum()) for k in Kr]
    bases = []
    off = 0
    for r in range(nregions):
        b = off + np.concatenate([[0], np.cumsum(Kr[r])[:-1]]).astype(np.int64)
        bases.append(b)
        off += Cr[r]
    C = off

    gid = (owner * TILES + tile_id) * nregions + key
    if tiebreak is None:
        tiebreak = np.arange(len(gid))
    order = np.lexsort((tiebreak, gid))
    gs = gid[order]
    starts = np.concatenate([[0], np.flatnonzero(np.diff(gs)) + 1])
    group_of = np.searchsorted(starts, np.arange(len(gs)), side="right") - 1
    pos = np.arange(len(gs)) - starts[group_of]

    base_chunk = np.empty(len(gid), np.int64)
    for r in range(nregions):
        m = key == r
        base_chunk[m] = bases[r][tile_id[m]]
    slot = np.empty(len(gs), np.int64)
    slot[order] = base_chunk[order] * 128 + pos
    return dict(K=Kr, C=Cr, bases=bases, Ctot=C, slot=slot)


def _preprocess(edge_index: np.ndarray, cfg=None):
    g = cfg or default_cfg()
    N, NCORES, NPC_REAL, TILES = g["N"], g["NCORES"], g["NPC_REAL"], g["TILES"]
    src = np.asarray(edge_index[0], np.int64)
    dst = np.asarray(edge_index[1], np.int64)
    deg = np.bincount(dst, minlength=N).astype(np.float64) + 1.0
    dinv_n = (1.0 / np.sqrt(deg)).astype(np.float32)

    # ---- layer 1: edges + self-loops, single region, slots carry x[src] ----
    selfn = np.arange(N, dtype=np.int64)
    src1 = np.concatenate([src, selfn])
    dst1 = np.concatenate([dst, selfn])
    own1 = dst1 // NPC_REAL
    dl1 = dst1 % NPC_REAL
    t1 = dl1 // 128
    it1 = dl1 % 128
    s1 = _schedule(own1, t1, np.zeros(len(src1), np.int64), 1, TILES,
                   NCORES, tiebreak=src1)
    C1 = s1["Ctot"]
    dstid1 = np.full((NCORES, C1 * 128), -1.0, np.float32)
    dstid1[own1, s1["slot"]] = it1
    dst1_t = np.ascontiguousarray(
        dstid1.reshape(NCORES, C1, 128).transpose(0, 2, 1)).astype(NPF16)
    srcof1 = np.full((NCORES, C1 * 128), -1, np.int64)
    srcof1[own1, s1["slot"]] = src1
    # dinv[dst] per layer-1 slot, folded host-side into the xgs slot values
    dinvdst1 = np.zeros((NCORES, C1 * 128), np.float32)
    dinvdst1[own1, s1["slot"]] = dinv_n[dst1]

    # ---- layer 2: edges only, regions = source node piece ----
    own2 = dst // NPC_REAL
    dl2 = dst % NPC_REAL
    t2 = dl2 // 128
    it2 = dl2 % 128
    srem = src % NPC_REAL
    offs = np.asarray(PIECE_OFF + [NPC], np.int64)
    reg2 = (np.searchsorted(offs, srem, side="right") - 1).astype(np.int64)
    hp = np.asarray(PIECE_H, np.int64)
    rowab = (src // NPC_REAL) * hp[reg2] + (srem - offs[reg2])
    s2 = _schedule(own2, t2, reg2, NPIECES, TILES, NCORES, tiebreak=rowab)
    C2 = s2["Ctot"]
    idx16 = np.zeros((NCORES, C2 * 128), np.int16)
    dstid2 = np.full((NCORES, C2 * 128), -1.0, np.float32)
    idx16[own2, s2["slot"]] = rowab.astype(np.int16)
    dstid2[own2, s2["slot"]] = it2
    idx_t = idx16.reshape(NCORES, C2 * 8, 16).transpose(0, 2, 1)
    idx_t = np.tile(idx_t, (1, 8, 1)).copy()                    # [8,128,C2*8]
    dst2_t = np.ascontiguousarray(
        dstid2.reshape(NCORES, C2, 128).transpose(0, 2, 1)).astype(NPF16)

    return dict(deg=deg, dinv_n=dinv_n, C1=C1, K1=s1["K"][0], B1=s1["bases"][0],
                dst1_t=dst1_t, srcof1=srcof1, dinvdst1=dinvdst1,
                C2=C2, K2=s2["K"], C2r=s2["C"], B2=s2["bases"],
                idx_t=idx_t, dst2_t=dst2_t)


def _waves(n_chunks: int, chunk0: int, wv: int):
    out, c = [], 0
    while c < n_chunks:
        n = min(wv, n_chunks - c)
        out.append((chunk0 + c, n))
        c += n
    return out


def _build_program(meta, cfg=None):
    g = cfg or default_cfg()
    IN, HID, OUT = g["IN"], g["HID"], g["OUT"]
    NCORES, TILES, NPC = g["NCORES"], g["TILES"], g["NPC"]
    C1, K1, B1 = meta["C1"], meta["K1"], meta["B1"]
    C2, K2, C2r, B2 = meta["C2"], meta["K2"], meta["C2r"], meta["B2"]

    nc = bacc.Bacc("TRN2", target_bir_lowering=False, debug=False,
                   num_devices=NCORES, num_swdge_queues=NQ)

    # ---- I/O ----
    xgs_d = nc.dram_tensor("xgs", [128, C1 * 128], F16, kind="ExternalInput")
    w1_d = nc.dram_tensor("W1", [IN, HID], F16, kind="ExternalInput")
    w2_d = nc.dram_tensor("W2", [HID, OUT], F16, kind="ExternalInput")
    b1_d = nc.dram_tensor("b1c", [HID, 1], F32, kind="ExternalInput")
    b2_d = nc.dram_tensor("b2c", [OUT, 1], F32, kind="ExternalInput")
    dinvrep_d = nc.dram_tensor("dinvrep", [128, NPC], F16,
                               kind="ExternalInput")
    ident_d = nc.dram_tensor("ident", [128, 128], F16, kind="ExternalInput")
    idx_d = nc.dram_tensor("idxt", [128, C2 * 8], mybir.dt.int16,
                           kind="ExternalInput")
    dst1_d = nc.dram_tensor("dstt1", [128, C1], F16, kind="ExternalInput")
    dst2_d = nc.dram_tensor("dstt2", [128, C2], F16, kind="ExternalInput")
    out_d = nc.dram_tensor("outT", [OUT, NPC], F32, kind="ExternalOutput")

    gdram = [nc.dram_tensor(f"gdram{p}", [PIECE_H[p], 128], F16)
             for p in range(NPIECES)]
    tables = [nc.dram_tensor(f"table{p}", [NCORES * PIECE_H[p], 128], F16,
                             addr_space="Shared") for p in range(NPIECES)]
    rg = [list(range(NCORES))]

    with tile.TileContext(nc) as tc:
        with (
            tc.tile_pool(name="const", bufs=1) as constp,
            tc.tile_pool(name="rdp", bufs=1) as rdp,
            tc.tile_pool(name="accp", bufs=1) as accp,
            tc.tile_pool(name="outp", bufs=3) as outp,
            tc.tile_pool(name="xw", bufs=3) as xwp,
            tc.tile_pool(name="gt", bufs=12) as gtp,
            tc.tile_pool(name="s1", bufs=3) as s1p,
            tc.tile_pool(name="s2", bufs=3) as s2p,
            tc.tile_pool(name="sx", bufs=3) as sxp,
            tc.tile_pool(name="rt", bufs=3) as rtp,
            tc.tile_pool(name="stg", bufs=3) as stgp,
            tc.tile_pool(name="psx", bufs=2, space="PSUM") as psxp,
            tc.tile_pool(name="pgem", bufs=2, space="PSUM") as pgemp,
            tc.tile_pool(name="pg2", bufs=1, space="PSUM") as pg2p,
            tc.tile_pool(name="psc", bufs=3, space="PSUM") as pscp,
        ):
            # ---- constants ----
            w1 = constp.tile([IN, HID], F16, tag="w1")
            nc.sync.dma_start(w1[:], w1_d[:, :])
            w2 = constp.tile([HID, OUT], F16, tag="w2")
            nc.sync.dma_start(w2[:], w2_d[:, :])
            b1 = constp.tile([HID, 1], F32, tag="b1")
            nc.sync.dma_start(b1[:], b1_d[:, :])
            b2 = constp.tile([OUT, 1], F32, tag="b2")
            nc.sync.dma_start(b2[:], b2_d[:, :])
            ident = constp.tile([128, 128], F16, tag="ident")
            nc.sync.dma_start(ident[:], ident_d[:, :])
            idxt = constp.tile([128, C2 * 8], mybir.dt.int16, tag="idxt")
            nc.sync.dma_start(idxt[:], idx_d[:, :])
            dstt1 = constp.tile([128, C1], F16, tag="dstt1")
            nc.sync.dma_start(dstt1[:], dst1_d[:, :])
            dstt2 = constp.tile([128, C2], F16, tag="dstt2")
            nc.sync.dma_start(dstt2[:], dst2_d[:, :])
            dinvrep = constp.tile([128, NPC], F16, tag="dinvrep")
            nc.sync.dma_start(dinvrep[:], dinvrep_d[:, :])

            iotat = constp.tile([128, WVS * 128], F16, tag="iotat")
            nc.gpsimd.iota(iotat[:].rearrange("p (k j) -> p k j", j=128),
                           [[0, WVS], [1, 128]], channel_multiplier=0,
                           allow_small_or_imprecise_dtypes=True)

            def build_s(eng, st, dstt, c0, n):
                eng.tensor_tensor(
                    st[:, :n * 128].rearrange("p (k j) -> p k j", j=128),
                    iotat[:, :n * 128].rearrange("p (k j) -> p k j", j=128),
                    dstt[:, c0:c0 + n].rearrange("p (k o) -> p k o", o=1)
                        .to_broadcast([128, n, 128]),
                    mybir.AluOpType.is_equal)

            # =================== LAYER 1 + piece staging ====================
            xgs_view = xgs_d.ap().rearrange("p (c f) -> p c f", f=128)
            l1_waves = _waves(C1, 0, WV)
            l1_sgrps = _waves(C1, 0, WVS)
            wave1, s1_tiles = {}, {}

            def ensure_wave1(wi):
                if wi in wave1:
                    return wave1[wi]
                c0, n = l1_waves[wi]
                t = xwp.tile([128, WV, 128], F16, tag="xw")
                nc.sync.dma_start(t[:, :n, :], xgs_view[:, c0:c0 + n, :])
                wave1[wi] = t
                return t

            def ensure_s1(wi):
                if wi in s1_tiles:
                    return s1_tiles[wi]
                c0, n = l1_sgrps[wi]
                st = s1p.tile([128, WVS * 128], F16, tag="s1")
                build_s(nc.vector, st, dstt1, c0, n)
                s1_tiles[wi] = st
                return st

            rd = rdp.tile([128, NPC], F16, tag="rd")       # h1 * dinv

            gviews = [gdram[p].ap().rearrange("(t p) f -> p t f", p=128)
                      for p in range(NPIECES)]  # f = OUT (64)
            piece_of = []
            for p in range(NPIECES):
                piece_of += [p] * PIECE_T[p]

            for t in range(TILES):
                nch = int(K1[t])
                sl = _ts(t, 128)
                psx = psxp.tile([IN, 128], F32, tag="psx")
                for k in range(nch):
                    ch = int(B1[t]) + k
                    xg = ensure_wave1(ch // WV)
                    sw = ensure_s1(ch // WVS)
                    pos, spos = ch % WV, ch % WVS
                    nc.tensor.matmul(
                        psx[:IN, :], xg[:, pos, :IN],
                        sw[:, spos * 128:(spos + 1) * 128],
                        start=(k == 0), stop=(k == nch - 1))
                sx = sxp.tile([IN, 128], F16, tag="sx")
                nc.scalar.copy(sx[:, :], psx[:IN, :])
                pz = pgemp.tile([128, 128], F32, tag="pgem")
                nc.tensor.matmul(pz[:HID, :], w1[:, :HID], sx[:, :],
                                 start=True, stop=True)
                # epilogue: rt = Relu(pz + b1) = h1 ; rd = h1*dinv
                rt = rtp.tile([128, 128], F16, tag="rt")
                nc.scalar.activation(
                    rt[:HID, :], pz[:HID, :],
                    mybir.ActivationFunctionType.Relu,
                    bias=b1[:HID, :], scale=1.0)
                nc.vector.tensor_tensor(
                    rd[:HID, sl], rt[:HID, :], dinvrep[:HID, sl],
                    mybir.AluOpType.mult)
                # stage this tile's node rows: (h1*dinv) @ W2, node-major
                ps2 = pg2p.tile([128, OUT], F32, tag="pg2")
                nc.tensor.matmul(ps2[:, :OUT], rd[:HID, sl], w2[:, :OUT],
                                 start=True, stop=True)
                stg = stgp.tile([128, OUT], F16, tag="stg")
                nc.scalar.copy(stg[:, :], ps2[:, :OUT])
                p = piece_of[t]
                nc.sync.dma_start(gviews[p][:, t - PIECE_T0[p], :OUT],
                                  stg[:, :])
                if t == PIECE_T0[p] + PIECE_T[p] - 1:
                    with tc.high_priority():
                        nc.gpsimd.collective_compute(
                            "AllGather", mybir.AluOpType.bypass,
                            replica_groups=rg,
                            ins=[gdram[p].ap()], outs=[tables[p].ap()])

            # =================== LAYER 2 gathers (prepare+trigger) ==========
            r_waves = [_waves(C2r[r], sum(C2r[:r]), WV) for r in range(NPIECES)]
            r_sgrps = [_waves(C2r[r], sum(C2r[:r]), WVS) for r in range(NPIECES)]
            wave2, s2_tiles = {}, {}
            qrr = [0]
            for r in range(NPIECES):
                with tc.tile_wait_until(0.15 + 0.03 * r):
                    for wi, (c0, n) in enumerate(r_waves[r]):
                        gt = gtp.tile([128, WV, 128], F16, tag="gt")
                        qn = qrr[0]
                        qrr[0] = (qn + 1) % NQ
                        nc.gpsimd.dma_gather(
                            gt[:, :n, :], tables[r][:, :],
                            idxt[:, c0 * 8:(c0 + n) * 8],
                            n * 128, n * 128, 128, single_packet=False,
                            queue_num=qn)
                        wave2[(r, wi)] = gt

            def ensure_s2(r, wi):
                key = (r, wi)
                if key in s2_tiles:
                    return s2_tiles[key]
                c0, n = r_sgrps[r][wi]
                st = s2p.tile([128, WVS * 128], F16, tag="s2")
                build_s(nc.vector, st, dstt2, c0, n)
                s2_tiles[key] = st
                return st

            acc = accp.tile([OUT, NPC], F16, tag="acc")

            # one pass per source piece; self term folded into pass 0
            for r in range(NPIECES):
                base0 = sum(C2r[:r])
                for t in range(TILES):
                    nch = int(K2[r][t])
                    sl = _ts(t, 128)
                    pscat = pscp.tile([OUT, 128], F32, tag="psc")
                    if r == 0:
                        nc.tensor.matmul(pscat[:OUT, :], w2[:, :OUT],
                                         rd[:HID, sl],
                                         start=True, stop=(nch == 0))
                    else:
                        nc.tensor.matmul(pscat[:OUT, :], ident[:OUT, :OUT],
                                         acc[:, sl],
                                         start=True, stop=(nch == 0))
                    for k in range(nch):
                        ch = int(B2[r][t]) + k
                        rel = ch - base0
                        gt = wave2[(r, rel // WV)]
                        sw = ensure_s2(r, rel // WVS)
                        pos, spos = rel % WV, rel % WVS
                        nc.tensor.matmul(
                            pscat[:OUT, :], gt[:, pos, :OUT],
                            sw[:, spos * 128:(spos + 1) * 128],
                            start=False, stop=(k == nch - 1))
                    if r < NPIECES - 1:
                        nc.scalar.copy(acc[:, sl], pscat[:OUT, :])
                    else:
                        tmp = rtp.tile([OUT, 128], F32, tag="ltmp")
                        nc.vector.tensor_tensor(
                            tmp[:, :], pscat[:OUT, :], dinvrep[:OUT, sl],
                            mybir.AluOpType.mult)
                        ot = outp.tile([OUT, 128], F32, tag="out")
                        nc.scalar.activation(
                            ot[:], tmp[:, :],
                            mybir.ActivationFunctionType.Relu,
                            bias=b2[:OUT, :], scale=1.0)
                        nc.sync.dma_start(out_d[:, sl], ot[:])

    nc.compile()
    return nc


def _host_inputs(inputs, meta, cfg=None):
    g = cfg or default_cfg()
    N, IN, HID, OUT = g["N"], g["IN"], g["HID"], g["OUT"]
    NCORES, NPC_REAL, NPC = g["NCORES"], g["NPC_REAL"], g["NPC"]
    x = np.asarray(inputs["x"], np.float32)
    W1 = np.asarray(inputs["W1"], np.float32)
    b1 = np.asarray(inputs["b1"], np.float32)
    W2 = np.asarray(inputs["W2"], np.float32)
    b2 = np.asarray(inputs["b2"], np.float32)
    C1 = meta["C1"]
    dinv_n = meta["dinv_n"]                                 # [N]
    xg = x * dinv_n[:, None]                                # [N, IN] f32

    ident = np.eye(128, dtype=NPF16)
    w1c = W1.astype(NPF16)
    w2c = W2.astype(NPF16)
    b1c = b1.reshape(HID, 1).astype(np.float32)
    b2c = b2.reshape(OUT, 1).astype(np.float32)

    in_maps = []
    for c in range(NCORES):
        srcof = meta["srcof1"][c]                           # [C1*128]
        xslots = np.zeros((C1 * 128, IN), NPF16)
        m = srcof >= 0
        xslots[m] = (xg[srcof[m]]
                     * meta["dinvdst1"][c][m, None]).astype(NPF16)
        xgs = np.ascontiguousarray(
            xslots.reshape(C1, 128, IN).transpose(1, 0, 2)
        ).reshape(128, C1 * IN)

        dl = np.ones(NPC, np.float32)
        dl[:NPC_REAL] = dinv_n[c * NPC_REAL:(c + 1) * NPC_REAL]
        dinvrep = np.tile(dl[None, :], (128, 1)).astype(NPF16)

        in_maps.append({
            "xgs": xgs, "W1": w1c, "W2": w2c, "b1c": b1c, "b2c": b2c,
            "dinvrep": dinvrep, "ident": ident,
            "idxt": meta["idx_t"][c],
            "dstt1": meta["dst1_t"][c], "dstt2": meta["dst2_t"][c],
        })
    return in_maps


def kernel(**inputs) -> np.ndarray:
    meta = _preprocess(np.asarray(inputs["edge_index"]))
    nc = _build_program(meta)
    in_maps = _host_inputs(inputs, meta)
    res = run_bass_kernel_spmd(nc, in_maps, list(range(NCORES)))
    out = np.empty((N, OUT), np.float32)
    for c in range(NCORES):
        out[c * NPC_REAL:(c + 1) * NPC_REAL] = \
            res.results[c]["outT"][:, :NPC_REAL].T
    return out
